# revision 31
# baseline (speedup 1.0000x reference)
"""Multi-head attention block (12 heads, N=2048, C=768) on 8 NeuronCores.

Sharding: core i = (batch b = i//2, head-group g = i%2). Each core computes
attention for 6 heads of one batch plus its slice of the output projection
(row-sharded Wproj); the host sums the two head-group partials per batch.

Per-core dataflow:
  QKV projection and output projection run in float32r (full-rate fp32,
  ~1.6e-4 matmul error). Attention (scores / exp / attn@V) runs in bf16.
  xT [768,2048] arrives host-transposed; QT/KT [384,2048] are column-major
  (head h lives at partitions (h%2)*64..+64 of tile h//2), V2 is token-major
  with a ones column per head (66th col = pad for even free size).

  Heads are processed in pairs (a=2j at PE rows 0-63, b=2j+1 at rows 64-127).
  Per (pair, 512-query chunk qs, key block k):
    S^T_a -> pss[:, 0:512], S^T_b -> pss[:, 512:1024]  (two matmuls in
      disjoint PE row groups, sharing one PSUM tile so the second has no
      semaphore wait and the pair runs concurrently in the array)
    es = exp(S/8) for both heads in ONE ACT instruction (PSUM->SBUF, bf16)
    U'_a += V2_a[k]^T @ es[:, 0:512], U'_b += V2_b[k]^T @ es[:, 512:1024]
      (PSUM [66,512] accumulated over k; row 64 = softmax denominator via
      the ones column; software-pipelined one k behind the scores)
  U rows are scaled by 1/denominator (DVE reciprocal + gpsimd partition
  broadcast + fused DVE multiply) into UT [384,2048] (f32r); odd heads take
  a small DMA hop to land at partitions 64-127.
  out = UT^T-chunks @ Wproj_rows (f32r, PSUM-accumulated), DMA out.
"""

import numpy as np
from contextlib import ExitStack

import concourse.bass as bass
import concourse.tile as tile
from concourse import bacc, mybir
from concourse.bass_utils import run_bass_kernel_spmd

N_CORES = 8
C = 768          # model dim
HG = 6           # heads per core
D = 64           # head dim
CHG = HG * D     # 384, per-group qkv width
CC = C // 128    # 6 contraction chunks
MT = CHG // 128  # 3 m-tiles for QT/KT
SCALE = 1.0 / 8.0

F32 = mybir.dt.float32
F32R = mybir.dt.float32r
BF16 = mybir.dt.bfloat16


def build(n_tok: int = 2048):
    NT = n_tok
    KB = NT // 128           # key blocks
    NQ = NT // 512           # 512-wide query chunks
    EXPF = mybir.ActivationFunctionType.Exp

    nc = bacc.Bacc("TRN2", target_bir_lowering=False, debug=False,
                   num_devices=N_CORES)

    xT = nc.dram_tensor("xT", [C, NT], F32R, kind="ExternalInput").ap()
    wq = nc.dram_tensor("wq", [C, CHG], F32R, kind="ExternalInput").ap()
    wk = nc.dram_tensor("wk", [C, CHG], F32R, kind="ExternalInput").ap()
    wv = nc.dram_tensor("wv", [C, CHG], F32R, kind="ExternalInput").ap()
    wp = nc.dram_tensor("wp", [CHG, C], F32R, kind="ExternalInput").ap()
    bqk = nc.dram_tensor("bqk", [128, 2 * MT], F32, kind="ExternalInput").ap()
    bv = nc.dram_tensor("bv", [1, CHG], F32, kind="ExternalInput").ap()
    out = nc.dram_tensor("out", [NT, C], F32, kind="ExternalOutput").ap()

    with tile.TileContext(nc) as tc, ExitStack() as ctx:
        wpool = ctx.enter_context(tc.tile_pool(name="w", bufs=1))
        perm = ctx.enter_context(tc.tile_pool(name="perm", bufs=1))
        psum = ctx.enter_context(tc.tile_pool(name="ps", bufs=2, space="PSUM"))
        psum_u = ctx.enter_context(tc.tile_pool(name="psu", bufs=4,
                                                space="PSUM"))

        # ---- persistent SBUF ----
        wq_t = [wpool.tile([128, CHG], F32R, tag=f"wq{c}", name=f"wq{c}")
                for c in range(CC)]
        wk_t = [wpool.tile([128, CHG], F32R, tag=f"wk{c}", name=f"wk{c}")
                for c in range(CC)]
        wv_t = [wpool.tile([128, CHG], F32R, tag=f"wv{c}", name=f"wv{c}")
                for c in range(CC)]
        wp_t = [wpool.tile([128, C], F32R, tag=f"wp{m}", name=f"wp{m}")
                for m in range(MT)]
        bqk_t = wpool.tile([128, 2 * MT], F32, tag="bqk")
        bv_row = wpool.tile([1, CHG], F32, tag="bvr")
        bv_bc = wpool.tile([128, CHG], F32, tag="bvb")

        QT = [perm.tile([128, NT], BF16, tag=f"qt{m}", name=f"qtt{m}")
              for m in range(MT)]
        KT = [perm.tile([128, NT], BF16, tag=f"kt{m}", name=f"ktt{m}")
              for m in range(MT)]
        V2 = [perm.tile([128, HG, 66], BF16, tag=f"v2{t}", name=f"v2t{t}")
              for t in range(KB)]
        UT = [perm.tile([128, NT], F32R, tag=f"ut{m}", name=f"utt{m}")
              for m in range(MT)]

        # ---- input DMA ----
        for c in range(CC):
            nc.sync.dma_start(wq_t[c][:], wq[c * 128:(c + 1) * 128, :])
            nc.sync.dma_start(wk_t[c][:], wk[c * 128:(c + 1) * 128, :])
            nc.sync.dma_start(wv_t[c][:], wv[c * 128:(c + 1) * 128, :])
        for m in range(MT):
            nc.sync.dma_start(wp_t[m][:], wp[m * 128:(m + 1) * 128, :])
        nc.sync.dma_start(bqk_t[:], bqk)
        nc.sync.dma_start(bv_row[0:1, :], bv[0:1, :])
        nc.gpsimd.partition_broadcast(bv_bc[:], bv_row[0:1, :])
        for t in range(KB):
            nc.vector.tensor_scalar(
                V2[t][:, :, 64:66],
                bv_bc[:, 0:12].rearrange("p (a b) -> p a b", a=HG),
                0.0, 1.0, mybir.AluOpType.mult, mybir.AluOpType.add)

        spool = ctx.enter_context(tc.tile_pool(name="es", bufs=10))
        rpool = ctx.enter_context(tc.tile_pool(name="rb", bufs=4))
        stpool = ctx.enter_context(tc.tile_pool(name="st", bufs=3))
        opool = ctx.enter_context(tc.tile_pool(name="ost", bufs=3))
        xpool = ctx.enter_context(tc.tile_pool(name="xt", bufs=1))

        # ---- QKV projection pieces ----
        xt = []
        for c in range(CC):
            xc = xpool.tile([128, NT], F32R, tag=f"x{c}", name=f"xt{c}")
            nc.sync.dma_start(xc[:], xT[c * 128:(c + 1) * 128, :])
            xt.append(xc)

        def qk_mtile(m):
            for wt, dst, bcol in ((wq_t, QT, m), (wk_t, KT, MT + m)):
                for n in range(NQ):
                    ps = psum.tile([128, 512], F32, tag="ps",
                                   name=f"psqk{m}_{n}")
                    for c in range(CC):
                        nc.tensor.matmul(
                            ps[:], wt[c][:, m * 128:(m + 1) * 128],
                            xt[c][:, n * 512:(n + 1) * 512],
                            start=(c == 0), stop=(c == CC - 1))
                    nc.vector.tensor_scalar_add(
                        dst[m][:, n * 512:(n + 1) * 512], ps[:],
                        bqk_t[:, bcol:bcol + 1])

        def v_tile(t):
            ps = psum_u.tile([128, CHG], F32, tag="psu", name=f"psv{t}")
            for c in range(CC):
                nc.tensor.matmul(ps[:], xt[c][:, t * 128:(t + 1) * 128],
                                 wv_t[c][:],
                                 start=(c == 0), stop=(c == CC - 1))
            nc.vector.tensor_add(
                V2[t][:, :, 0:64],
                ps[:].rearrange("p (h d) -> p h d", h=HG),
                bv_bc[:].rearrange("p (h d) -> p h d", h=HG))

        def v_proj(ts):
            for t in ts:
                v_tile(t)

        # ---- attention pieces ----
        def attn_pair(qs, j, k_hook=None):
            ha, hb = 2 * j, 2 * j + 1
            q0 = qs * 512
            psu = [psum_u.tile([128, 512], F32, tag="psu",
                               name=f"psu{j}_{qs}_{i}") for i in range(2)]

            def emit_pv(k, es):
                for i, h in enumerate((ha, hb)):
                    nc.tensor.matmul(
                        psu[i][0:66, :], V2[k][:, h, :],
                        es[:, i * 512:(i + 1) * 512],
                        start=(k == 0), stop=(k == KB - 1))

            prev = None
            for k in range(KB):
                if k_hook is not None:
                    k_hook(k)
                pss = psum.tile([128, 1024], F32, tag="ps",
                                name=f"pss{j}_{qs}_{k}")
                # head a in PE rows 0-63 -> pss cols 0-511, head b in
                # rows 64-127 -> cols 512-1023; one shared PSUM tile so
                # the second matmul has no semaphore wait and the pair
                # runs concurrently in disjoint row groups.
                for i, off in ((0, 0), (1, 64)):
                    nc.tensor.matmul(
                        pss[:, i * 512:(i + 1) * 512],
                        KT[j][off:off + 64, k * 128:(k + 1) * 128],
                        QT[j][off:off + 64, q0:q0 + 512],
                        start=True, stop=True)
                es = spool.tile([128, 1024], BF16, tag="es",
                                name=f"es{j}_{qs}_{k}")
                nc.scalar.activation(es[:], pss[:], EXPF, scale=SCALE)
                if prev is not None:
                    emit_pv(k - 1, prev)
                prev = es
            emit_pv(KB - 1, prev)

            # denominators -> reciprocal -> replicate -> scale U rows
            for i, off in ((0, 0), (1, 64)):
                rb = rpool.tile([128, 512], F32, tag="rb",
                                name=f"rb{j}_{qs}_{i}")
                rc = rpool.tile([128, 512], F32, tag="rb",
                                name=f"rc{j}_{qs}_{i}")
                rg = rpool.tile([128, 8], F32, tag="rg",
                                name=f"rg{j}_{qs}_{i}")
                # denom row (512 on one partition) -> spread over 128
                # partitions so the exact reciprocal runs 4 elems/lane
                nc.vector.tensor_copy(rb[64:65, :], psu[i][64:65, :])
                nc.sync.dma_start(rg[:, 0:4], rb[64:65, :])
                nc.vector.reciprocal(rg[:, 4:8], rg[:, 0:4])
                nc.sync.dma_start(rc[0:1, :], rg[:, 4:8])
                nc.gpsimd.partition_broadcast(rc[0:64, :], rc[0:1, :])
                if off == 0:
                    nc.vector.tensor_mul(UT[j][0:64, q0:q0 + 512],
                                         psu[i][0:64, :], rc[0:64, :])
                else:
                    st = stpool.tile([64, 512], F32R, tag="st",
                                     name=f"st{j}_{qs}_{i}")
                    nc.vector.tensor_mul(st[0:64, :], psu[i][0:64, :],
                                         rc[0:64, :])
                    nc.sync.dma_start(UT[j][64:128, q0:q0 + 512],
                                      st[0:64, :])

        def proj_qs(qs):
            for qb in range(qs * 4, qs * 4 + 4):
                plo = psum_u.tile([128, 512], F32, tag="psu", name=f"pl{qb}")
                phi = psum_u.tile([128, 512], F32, tag="psu", name=f"ph{qb}")
                for m in range(MT):
                    lhsT = UT[m][:, qb * 128:(qb + 1) * 128]
                    nc.tensor.matmul(plo[:, 0:512], lhsT, wp_t[m][:, 0:512],
                                     start=(m == 0), stop=(m == MT - 1))
                    nc.tensor.matmul(phi[:, 0:256], lhsT,
                                     wp_t[m][:, 512:768],
                                     start=(m == 0), stop=(m == MT - 1))
                ot = opool.tile([128, C], F32, tag="ost", name=f"ot{qb}")
                nc.vector.tensor_copy(ot[:, 0:512], plo[:, 0:512])
                nc.vector.tensor_copy(ot[:, 512:768], phi[:, 0:256])
                nc.sync.dma_start(out[qb * 128:(qb + 1) * 128, :], ot[:])

        # ---- emission schedule: pipeline QKV m-tiles into attention ----
        VHEAD = min(6, KB)
        qk_mtile(0)
        v_proj(range(VHEAD))
        attn_pair(0, 0, k_hook=lambda k: v_tile(k) if k >= VHEAD else None)
        qk_mtile(1)
        attn_pair(0, 1)
        qk_mtile(2)
        attn_pair(0, 2)
        for qs in range(1, NQ):
            attn_pair(qs, 0)
            proj_qs(qs - 1)
            attn_pair(qs, 1)
            attn_pair(qs, 2)
        proj_qs(NQ - 1)

    nc.compile()
    return nc


_built = {}


def _get_nc(n_tok=2048):
    if n_tok not in _built:
        _built[n_tok] = build(n_tok)
    return _built[n_tok]


def make_in_maps(x, Wqkv, bqkv, Wproj):
    B, NT, _ = x.shape
    x = np.ascontiguousarray(np.asarray(x, dtype=np.float32))
    Wqkv = np.asarray(Wqkv, dtype=np.float32)
    bqkv = np.asarray(bqkv, dtype=np.float32)
    Wproj = np.asarray(Wproj, dtype=np.float32)
    in_maps = []
    for i in range(N_CORES):
        b, g = i // 2, i % 2
        s = g * CHG
        bq = bqkv[s:s + CHG].reshape(MT, 128).T
        bk = bqkv[C + s:C + s + CHG].reshape(MT, 128).T
        in_maps.append({
            "xT": np.ascontiguousarray(x[b].T),
            "wq": np.ascontiguousarray(Wqkv[:, s:s + CHG]),
            "wk": np.ascontiguousarray(Wqkv[:, C + s:C + s + CHG]),
            "wv": np.ascontiguousarray(Wqkv[:, 2 * C + s:2 * C + s + CHG]),
            "wp": np.ascontiguousarray(Wproj[s:s + CHG, :]),
            "bqk": np.ascontiguousarray(
                np.concatenate([bq, bk], axis=1)).astype(np.float32),
            "bv": np.ascontiguousarray(
                bqkv[2 * C + s:2 * C + s + CHG][None, :]).astype(np.float32),
        })
    return in_maps


def gather(results, bproj, B, NT):
    parts = [results[i]["out"] for i in range(N_CORES)]
    out = np.stack([parts[2 * b] + parts[2 * b + 1] for b in range(B)])
    return (out + np.asarray(bproj, np.float32)[None, None, :]).astype(np.float32)


def kernel(x, Wqkv, bqkv, Wproj, bproj, _trace=False):
    x = np.asarray(x)
    B, NT, _ = x.shape
    nc = _get_nc(NT)
    in_maps = make_in_maps(x, Wqkv, bqkv, Wproj)
    res = run_bass_kernel_spmd(nc, in_maps, core_ids=list(range(N_CORES)),
                               trace=_trace)
    out = gather(res.results, bproj, B, NT)
    if _trace:
        return out, res
    return out


# revision 32
# speedup vs baseline: 1.0005x; 1.0005x over previous
"""Multi-head attention block (12 heads, N=2048, C=768) on 8 NeuronCores.

Sharding: core i = (batch b = i//2, head-group g = i%2). Each core computes
attention for 6 heads of one batch plus its slice of the output projection
(row-sharded Wproj); the host sums the two head-group partials per batch.

Per-core dataflow:
  QKV projection and output projection run in float32r (full-rate fp32,
  ~1.6e-4 matmul error). Attention (scores / exp / attn@V) runs in bf16.
  xT [768,2048] arrives host-transposed; QT/KT [384,2048] are column-major
  (head h lives at partitions (h%2)*64..+64 of tile h//2), V2 is token-major
  with a ones column per head (66th col = pad for even free size).

  Heads are processed in pairs (a=2j at PE rows 0-63, b=2j+1 at rows 64-127).
  Per (pair, 512-query chunk qs, key block k):
    S^T_a -> pss[:, 0:512], S^T_b -> pss[:, 512:1024]  (two matmuls in
      disjoint PE row groups, sharing one PSUM tile so the second has no
      semaphore wait and the pair runs concurrently in the array)
    es = exp(S/8) for both heads in ONE ACT instruction (PSUM->SBUF, bf16)
    U'_a += V2_a[k]^T @ es[:, 0:512], U'_b += V2_b[k]^T @ es[:, 512:1024]
      (PSUM [66,512] accumulated over k; row 64 = softmax denominator via
      the ones column; software-pipelined one k behind the scores)
  U rows are scaled by 1/denominator (DVE reciprocal + gpsimd partition
  broadcast + fused DVE multiply) into UT [384,2048] (f32r); odd heads take
  a small DMA hop to land at partitions 64-127.
  out = UT^T-chunks @ Wproj_rows (f32r, PSUM-accumulated), DMA out.
"""

import numpy as np
from contextlib import ExitStack

import concourse.bass as bass
import concourse.tile as tile
from concourse import bacc, mybir
from concourse.bass_utils import run_bass_kernel_spmd

N_CORES = 8
C = 768          # model dim
HG = 6           # heads per core
D = 64           # head dim
CHG = HG * D     # 384, per-group qkv width
CC = C // 128    # 6 contraction chunks
MT = CHG // 128  # 3 m-tiles for QT/KT
SCALE = 1.0 / 8.0

F32 = mybir.dt.float32
F32R = mybir.dt.float32r
BF16 = mybir.dt.bfloat16


def build(n_tok: int = 2048):
    NT = n_tok
    KB = NT // 128           # key blocks
    NQ = NT // 512           # 512-wide query chunks
    EXPF = mybir.ActivationFunctionType.Exp

    nc = bacc.Bacc("TRN2", target_bir_lowering=False, debug=False,
                   num_devices=N_CORES)

    xT = nc.dram_tensor("xT", [C, NT], F32R, kind="ExternalInput").ap()
    wq = nc.dram_tensor("wq", [C, CHG], F32R, kind="ExternalInput").ap()
    wk = nc.dram_tensor("wk", [C, CHG], F32R, kind="ExternalInput").ap()
    wv = nc.dram_tensor("wv", [C, CHG], F32R, kind="ExternalInput").ap()
    wp = nc.dram_tensor("wp", [CHG, C], F32R, kind="ExternalInput").ap()
    bqk = nc.dram_tensor("bqk", [128, 2 * MT], F32, kind="ExternalInput").ap()
    bv = nc.dram_tensor("bv", [1, CHG], F32, kind="ExternalInput").ap()
    out = nc.dram_tensor("out", [NT, C], F32, kind="ExternalOutput").ap()

    with tile.TileContext(nc) as tc, ExitStack() as ctx:
        wpool = ctx.enter_context(tc.tile_pool(name="w", bufs=1))
        perm = ctx.enter_context(tc.tile_pool(name="perm", bufs=1))
        psum = ctx.enter_context(tc.tile_pool(name="ps", bufs=2, space="PSUM"))
        psum_u = ctx.enter_context(tc.tile_pool(name="psu", bufs=4,
                                                space="PSUM"))

        # ---- persistent SBUF ----
        wq_t = [wpool.tile([128, CHG], F32R, tag=f"wq{c}", name=f"wq{c}")
                for c in range(CC)]
        wk_t = [wpool.tile([128, CHG], F32R, tag=f"wk{c}", name=f"wk{c}")
                for c in range(CC)]
        wv_t = [wpool.tile([128, CHG], F32R, tag=f"wv{c}", name=f"wv{c}")
                for c in range(CC)]
        wp_t = [wpool.tile([128, C], F32R, tag=f"wp{m}", name=f"wp{m}")
                for m in range(MT)]
        bqk_t = wpool.tile([128, 2 * MT], F32, tag="bqk")
        bv_row = wpool.tile([1, CHG], F32, tag="bvr")
        bv_bc = wpool.tile([128, CHG], F32, tag="bvb")

        QT = [perm.tile([128, NT], BF16, tag=f"qt{m}", name=f"qtt{m}")
              for m in range(MT)]
        KT = [perm.tile([128, NT], BF16, tag=f"kt{m}", name=f"ktt{m}")
              for m in range(MT)]
        V2 = [perm.tile([128, HG, 66], BF16, tag=f"v2{t}", name=f"v2t{t}")
              for t in range(KB)]
        UT = [perm.tile([128, NT], F32R, tag=f"ut{m}", name=f"utt{m}")
              for m in range(MT)]

        # ---- input DMA ----
        for c in range(CC):
            nc.sync.dma_start(wq_t[c][:], wq[c * 128:(c + 1) * 128, :])
            nc.sync.dma_start(wk_t[c][:], wk[c * 128:(c + 1) * 128, :])
            nc.sync.dma_start(wv_t[c][:], wv[c * 128:(c + 1) * 128, :])
        for m in range(MT):
            nc.sync.dma_start(wp_t[m][:], wp[m * 128:(m + 1) * 128, :])
        nc.sync.dma_start(bqk_t[:], bqk)
        nc.sync.dma_start(bv_row[0:1, :], bv[0:1, :])
        nc.gpsimd.partition_broadcast(bv_bc[:], bv_row[0:1, :])
        for t in range(KB):
            nc.vector.tensor_scalar(
                V2[t][:, :, 64:66],
                bv_bc[:, 0:12].rearrange("p (a b) -> p a b", a=HG),
                0.0, 1.0, mybir.AluOpType.mult, mybir.AluOpType.add)

        spool = ctx.enter_context(tc.tile_pool(name="es", bufs=10))
        rpool = ctx.enter_context(tc.tile_pool(name="rb", bufs=4))
        stpool = ctx.enter_context(tc.tile_pool(name="st", bufs=3))
        opool = ctx.enter_context(tc.tile_pool(name="ost", bufs=3))
        xpool = ctx.enter_context(tc.tile_pool(name="xt", bufs=1))

        # ---- QKV projection pieces ----
        xt = []
        for c in range(CC):
            xc = xpool.tile([128, NT], F32R, tag=f"x{c}", name=f"xt{c}")
            nc.sync.dma_start(xc[:], xT[c * 128:(c + 1) * 128, :])
            xt.append(xc)

        def qk_group(m, ti, n, pool):
            wt = (wq_t, wk_t)[ti]
            dst = (QT, KT)[ti]
            tg = "ps" if pool is psum else "psu"
            ps = pool.tile([128, 512], F32, tag=tg, name=f"psg{ti}_{m}_{n}")
            for c in range(CC):
                nc.tensor.matmul(
                    ps[:], wt[c][:, m * 128:(m + 1) * 128],
                    xt[c][:, n * 512:(n + 1) * 512],
                    start=(c == 0), stop=(c == CC - 1))
            nc.vector.tensor_scalar_add(
                dst[m][:, n * 512:(n + 1) * 512], ps[:],
                bqk_t[:, m + MT * ti:m + MT * ti + 1])

        def qk_mtile(m):
            for ti in (0, 1):
                for n in range(NQ):
                    qk_group(m, ti, n, psum)

        def v_tile(t):
            ps = psum_u.tile([128, CHG], F32, tag="psu", name=f"psv{t}")
            for c in range(CC):
                nc.tensor.matmul(ps[:], xt[c][:, t * 128:(t + 1) * 128],
                                 wv_t[c][:],
                                 start=(c == 0), stop=(c == CC - 1))
            nc.vector.tensor_add(
                V2[t][:, :, 0:64],
                ps[:].rearrange("p (h d) -> p h d", h=HG),
                bv_bc[:].rearrange("p (h d) -> p h d", h=HG))

        def v_proj(ts):
            for t in ts:
                v_tile(t)

        # ---- attention pieces ----
        def attn_pair(qs, j, k_hook=None):
            ha, hb = 2 * j, 2 * j + 1
            q0 = qs * 512
            psu = [psum_u.tile([128, 512], F32, tag="psu",
                               name=f"psu{j}_{qs}_{i}") for i in range(2)]

            def emit_pv(k, es):
                for i, h in enumerate((ha, hb)):
                    nc.tensor.matmul(
                        psu[i][0:66, :], V2[k][:, h, :],
                        es[:, i * 512:(i + 1) * 512],
                        start=(k == 0), stop=(k == KB - 1))

            prev = None
            for k in range(KB):
                pss = psum.tile([128, 1024], F32, tag="ps",
                                name=f"pss{j}_{qs}_{k}")
                # head a in PE rows 0-63 -> pss cols 0-511, head b in
                # rows 64-127 -> cols 512-1023; one shared PSUM tile so
                # the second matmul has no semaphore wait and the pair
                # runs concurrently in disjoint row groups.
                for i, off in ((0, 0), (1, 64)):
                    nc.tensor.matmul(
                        pss[:, i * 512:(i + 1) * 512],
                        KT[j][off:off + 64, k * 128:(k + 1) * 128],
                        QT[j][off:off + 64, q0:q0 + 512],
                        start=True, stop=True)
                es = spool.tile([128, 1024], BF16, tag="es",
                                name=f"es{j}_{qs}_{k}")
                nc.scalar.activation(es[:], pss[:], EXPF, scale=SCALE)
                if k_hook is not None:
                    k_hook(k)
                if prev is not None:
                    emit_pv(k - 1, prev)
                prev = es
            emit_pv(KB - 1, prev)

            # denominators -> reciprocal -> replicate -> scale U rows
            for i, off in ((0, 0), (1, 64)):
                rb = rpool.tile([128, 512], F32, tag="rb",
                                name=f"rb{j}_{qs}_{i}")
                rc = rpool.tile([128, 512], F32, tag="rb",
                                name=f"rc{j}_{qs}_{i}")
                rg = rpool.tile([128, 8], F32, tag="rg",
                                name=f"rg{j}_{qs}_{i}")
                # denom row (512 on one partition) -> spread over 128
                # partitions so the exact reciprocal runs 4 elems/lane
                nc.vector.tensor_copy(rb[64:65, :], psu[i][64:65, :])
                nc.sync.dma_start(rg[:, 0:4], rb[64:65, :])
                nc.vector.reciprocal(rg[:, 4:8], rg[:, 0:4])
                nc.sync.dma_start(rc[0:1, :], rg[:, 4:8])
                nc.gpsimd.partition_broadcast(rc[0:64, :], rc[0:1, :])
                if off == 0:
                    nc.vector.tensor_mul(UT[j][0:64, q0:q0 + 512],
                                         psu[i][0:64, :], rc[0:64, :])
                else:
                    st = stpool.tile([64, 512], F32R, tag="st",
                                     name=f"st{j}_{qs}_{i}")
                    nc.vector.tensor_mul(st[0:64, :], psu[i][0:64, :],
                                         rc[0:64, :])
                    nc.sync.dma_start(UT[j][64:128, q0:q0 + 512],
                                      st[0:64, :])

        def proj_qs(qs):
            for qb in range(qs * 4, qs * 4 + 4):
                plo = psum_u.tile([128, 512], F32, tag="psu", name=f"pl{qb}")
                phi = psum_u.tile([128, 512], F32, tag="psu", name=f"ph{qb}")
                for m in range(MT):
                    lhsT = UT[m][:, qb * 128:(qb + 1) * 128]
                    nc.tensor.matmul(plo[:, 0:512], lhsT, wp_t[m][:, 0:512],
                                     start=(m == 0), stop=(m == MT - 1))
                    nc.tensor.matmul(phi[:, 0:256], lhsT,
                                     wp_t[m][:, 512:768],
                                     start=(m == 0), stop=(m == MT - 1))
                ot = opool.tile([128, C], F32, tag="ost", name=f"ot{qb}")
                nc.vector.tensor_copy(ot[:, 0:512], plo[:, 0:512])
                nc.vector.tensor_copy(ot[:, 512:768], phi[:, 0:256])
                nc.sync.dma_start(out[qb * 128:(qb + 1) * 128, :], ot[:])

        # ---- emission schedule: pipeline QKV m-tiles into attention ----
        VHEAD = min(2, KB)
        qk_group(0, 0, 0, psum)
        qk_group(0, 1, 0, psum)
        v_proj(range(VHEAD))

        def hook0(k):
            n = k + 1
            if n < NQ:
                qk_group(0, 0, n, psum_u)
                qk_group(0, 1, n, psum_u)
            t = k + VHEAD
            if VHEAD <= t < KB:
                v_tile(t)

        attn_pair(0, 0, k_hook=hook0)
        qk_mtile(1)
        attn_pair(0, 1)
        qk_mtile(2)
        attn_pair(0, 2)
        for qs in range(1, NQ):
            attn_pair(qs, 0)
            proj_qs(qs - 1)
            attn_pair(qs, 1)
            attn_pair(qs, 2)
        proj_qs(NQ - 1)

    nc.compile()
    return nc


_built = {}


def _get_nc(n_tok=2048):
    if n_tok not in _built:
        _built[n_tok] = build(n_tok)
    return _built[n_tok]


def make_in_maps(x, Wqkv, bqkv, Wproj):
    B, NT, _ = x.shape
    x = np.ascontiguousarray(np.asarray(x, dtype=np.float32))
    Wqkv = np.asarray(Wqkv, dtype=np.float32)
    bqkv = np.asarray(bqkv, dtype=np.float32)
    Wproj = np.asarray(Wproj, dtype=np.float32)
    in_maps = []
    for i in range(N_CORES):
        b, g = i // 2, i % 2
        s = g * CHG
        bq = bqkv[s:s + CHG].reshape(MT, 128).T
        bk = bqkv[C + s:C + s + CHG].reshape(MT, 128).T
        in_maps.append({
            "xT": np.ascontiguousarray(x[b].T),
            "wq": np.ascontiguousarray(Wqkv[:, s:s + CHG]),
            "wk": np.ascontiguousarray(Wqkv[:, C + s:C + s + CHG]),
            "wv": np.ascontiguousarray(Wqkv[:, 2 * C + s:2 * C + s + CHG]),
            "wp": np.ascontiguousarray(Wproj[s:s + CHG, :]),
            "bqk": np.ascontiguousarray(
                np.concatenate([bq, bk], axis=1)).astype(np.float32),
            "bv": np.ascontiguousarray(
                bqkv[2 * C + s:2 * C + s + CHG][None, :]).astype(np.float32),
        })
    return in_maps


def gather(results, bproj, B, NT):
    parts = [results[i]["out"] for i in range(N_CORES)]
    out = np.stack([parts[2 * b] + parts[2 * b + 1] for b in range(B)])
    return (out + np.asarray(bproj, np.float32)[None, None, :]).astype(np.float32)


def kernel(x, Wqkv, bqkv, Wproj, bproj, _trace=False):
    x = np.asarray(x)
    B, NT, _ = x.shape
    nc = _get_nc(NT)
    in_maps = make_in_maps(x, Wqkv, bqkv, Wproj)
    res = run_bass_kernel_spmd(nc, in_maps, core_ids=list(range(N_CORES)),
                               trace=_trace)
    out = gather(res.results, bproj, B, NT)
    if _trace:
        return out, res
    return out


# revision 35
# speedup vs baseline: 1.0038x; 1.0033x over previous
"""Multi-head attention block (12 heads, N=2048, C=768) on 8 NeuronCores.

Sharding: core i = (batch b = i//2, head-group g = i%2). Each core computes
attention for 6 heads of one batch plus its slice of the output projection
(row-sharded Wproj); the host sums the two head-group partials per batch.

Per-core dataflow:
  QKV projection and output projection run in float32r (full-rate fp32,
  ~1.6e-4 matmul error). Attention (scores / exp / attn@V) runs in bf16.
  xT [768,2048] arrives host-transposed; QT/KT [384,2048] are column-major
  (head h lives at partitions (h%2)*64..+64 of tile h//2), V2 is token-major
  with a ones column per head (66th col = pad for even free size).

  Heads are processed in pairs (a=2j at PE rows 0-63, b=2j+1 at rows 64-127).
  Per (pair, 512-query chunk qs, key block k):
    S^T_a -> pss[:, 0:512], S^T_b -> pss[:, 512:1024]  (two matmuls in
      disjoint PE row groups, sharing one PSUM tile so the second has no
      semaphore wait and the pair runs concurrently in the array)
    es = exp(S/8) for both heads in ONE ACT instruction (PSUM->SBUF, bf16)
    U'_a += V2_a[k]^T @ es[:, 0:512], U'_b += V2_b[k]^T @ es[:, 512:1024]
      (PSUM [66,512] accumulated over k; row 64 = softmax denominator via
      the ones column; software-pipelined one k behind the scores)
  U rows are scaled by 1/denominator (DVE reciprocal + gpsimd partition
  broadcast + fused DVE multiply) into UT [384,2048] (f32r); odd heads take
  a small DMA hop to land at partitions 64-127.
  out = UT^T-chunks @ Wproj_rows (f32r, PSUM-accumulated), DMA out.
"""

import numpy as np
from contextlib import ExitStack

import concourse.bass as bass
import concourse.tile as tile
from concourse import bacc, mybir
from concourse.bass_utils import run_bass_kernel_spmd

N_CORES = 8
C = 768          # model dim
HG = 6           # heads per core
D = 64           # head dim
CHG = HG * D     # 384, per-group qkv width
CC = C // 128    # 6 contraction chunks
MT = CHG // 128  # 3 m-tiles for QT/KT
SCALE = 1.0 / 8.0

F32 = mybir.dt.float32
F32R = mybir.dt.float32r
BF16 = mybir.dt.bfloat16


def build(n_tok: int = 2048):
    NT = n_tok
    KB = NT // 128           # key blocks
    NQ = NT // 512           # 512-wide query chunks
    EXPF = mybir.ActivationFunctionType.Exp

    nc = bacc.Bacc("TRN2", target_bir_lowering=False, debug=False,
                   num_devices=N_CORES)

    xT = nc.dram_tensor("xT", [C, NT], F32R, kind="ExternalInput").ap()
    wq = nc.dram_tensor("wq", [C, CHG], F32R, kind="ExternalInput").ap()
    wk = nc.dram_tensor("wk", [C, CHG], F32R, kind="ExternalInput").ap()
    wv = nc.dram_tensor("wv", [C, CHG], F32R, kind="ExternalInput").ap()
    wp = nc.dram_tensor("wp", [CHG, C], F32R, kind="ExternalInput").ap()
    bqk = nc.dram_tensor("bqk", [128, 2 * MT], F32, kind="ExternalInput").ap()
    bv = nc.dram_tensor("bv", [1, CHG], F32, kind="ExternalInput").ap()
    out = nc.dram_tensor("out", [NT, C], F32, kind="ExternalOutput").ap()

    with tile.TileContext(nc) as tc, ExitStack() as ctx:
        wpool = ctx.enter_context(tc.tile_pool(name="w", bufs=1))
        perm = ctx.enter_context(tc.tile_pool(name="perm", bufs=1))
        psum = ctx.enter_context(tc.tile_pool(name="ps", bufs=2, space="PSUM"))
        psum_u = ctx.enter_context(tc.tile_pool(name="psu", bufs=4,
                                                space="PSUM"))

        # ---- persistent SBUF ----
        wq_t = [wpool.tile([128, CHG], F32R, tag=f"wq{c}", name=f"wq{c}")
                for c in range(CC)]
        wk_t = [wpool.tile([128, CHG], F32R, tag=f"wk{c}", name=f"wk{c}")
                for c in range(CC)]
        wv_t = [wpool.tile([128, CHG], F32R, tag=f"wv{c}", name=f"wv{c}")
                for c in range(CC)]
        wp_t = [wpool.tile([128, C], F32R, tag=f"wp{m}", name=f"wp{m}")
                for m in range(MT)]
        bqk_t = wpool.tile([128, 2 * MT], F32, tag="bqk")
        bv_row = wpool.tile([1, CHG], F32, tag="bvr")
        bv_bc = wpool.tile([128, CHG], F32, tag="bvb")

        QT = [perm.tile([128, NT], BF16, tag=f"qt{m}", name=f"qtt{m}")
              for m in range(MT)]
        KT = [perm.tile([128, NT], BF16, tag=f"kt{m}", name=f"ktt{m}")
              for m in range(MT)]
        V2 = [perm.tile([128, HG, 66], BF16, tag=f"v2{t}", name=f"v2t{t}")
              for t in range(KB)]
        UT = [perm.tile([128, NT], F32R, tag=f"ut{m}", name=f"utt{m}")
              for m in range(MT)]

        # ---- input DMA ----
        for c in range(CC):
            nc.sync.dma_start(wq_t[c][:], wq[c * 128:(c + 1) * 128, :])
            nc.sync.dma_start(wk_t[c][:], wk[c * 128:(c + 1) * 128, :])
            nc.sync.dma_start(wv_t[c][:], wv[c * 128:(c + 1) * 128, :])
        for m in range(MT):
            nc.sync.dma_start(wp_t[m][:], wp[m * 128:(m + 1) * 128, :])
        nc.sync.dma_start(bqk_t[:], bqk)
        nc.sync.dma_start(bv_row[0:1, :], bv[0:1, :])
        nc.gpsimd.partition_broadcast(bv_bc[:], bv_row[0:1, :])
        for t in range(KB):
            nc.vector.tensor_scalar(
                V2[t][:, :, 64:66],
                bv_bc[:, 0:12].rearrange("p (a b) -> p a b", a=HG),
                0.0, 1.0, mybir.AluOpType.mult, mybir.AluOpType.add)

        spool = ctx.enter_context(tc.tile_pool(name="es", bufs=10))
        rpool = ctx.enter_context(tc.tile_pool(name="rb", bufs=4))
        stpool = ctx.enter_context(tc.tile_pool(name="st", bufs=3))
        opool = ctx.enter_context(tc.tile_pool(name="ost", bufs=3))
        xpool = ctx.enter_context(tc.tile_pool(name="xt", bufs=1))

        # ---- QKV projection pieces ----
        xt = []
        for c in range(CC):
            xc = xpool.tile([128, NT], F32R, tag=f"x{c}", name=f"xt{c}")
            nc.sync.dma_start(xc[:], xT[c * 128:(c + 1) * 128, :])
            xt.append(xc)

        def qk_mtile(m):
            for wt, dst, bcol in ((wq_t, QT, m), (wk_t, KT, MT + m)):
                for n in range(NQ):
                    ps = psum.tile([128, 512], F32, tag="ps",
                                   name=f"psqk{m}_{n}")
                    for c in range(CC):
                        nc.tensor.matmul(
                            ps[:], wt[c][:, m * 128:(m + 1) * 128],
                            xt[c][:, n * 512:(n + 1) * 512],
                            start=(c == 0), stop=(c == CC - 1))
                    nc.vector.tensor_scalar_add(
                        dst[m][:, n * 512:(n + 1) * 512], ps[:],
                        bqk_t[:, bcol:bcol + 1])

        def v_tile(t):
            ps = psum_u.tile([128, CHG], F32, tag="psu", name=f"psv{t}")
            for c in range(CC):
                nc.tensor.matmul(ps[:], xt[c][:, t * 128:(t + 1) * 128],
                                 wv_t[c][:],
                                 start=(c == 0), stop=(c == CC - 1))
            nc.vector.tensor_add(
                V2[t][:, :, 0:64],
                ps[:].rearrange("p (h d) -> p h d", h=HG),
                bv_bc[:].rearrange("p (h d) -> p h d", h=HG))

        def v_proj(ts):
            for t in ts:
                v_tile(t)

        # ---- attention pieces ----
        def attn_pair(qs, j, k_hook=None):
            ha, hb = 2 * j, 2 * j + 1
            q0 = qs * 512
            psu = [psum_u.tile([128, 512], F32, tag="psu",
                               name=f"psu{j}_{qs}_{i}") for i in range(2)]

            def emit_pv(k, es):
                for i, h in enumerate((ha, hb)):
                    nc.tensor.matmul(
                        psu[i][0:66, :], V2[k][:, h, :],
                        es[:, i * 512:(i + 1) * 512],
                        start=(k == 0), stop=(k == KB - 1))

            prev = None
            for k in range(KB):
                if k_hook is not None:
                    k_hook(k)
                pss = psum.tile([128, 1024], F32, tag="ps",
                                name=f"pss{j}_{qs}_{k}")
                # head a in PE rows 0-63 -> pss cols 0-511, head b in
                # rows 64-127 -> cols 512-1023; one shared PSUM tile so
                # the second matmul has no semaphore wait and the pair
                # runs concurrently in disjoint row groups.
                for i, off in ((0, 0), (1, 64)):
                    nc.tensor.matmul(
                        pss[:, i * 512:(i + 1) * 512],
                        KT[j][off:off + 64, k * 128:(k + 1) * 128],
                        QT[j][off:off + 64, q0:q0 + 512],
                        start=True, stop=True)
                es = spool.tile([128, 1024], BF16, tag="es",
                                name=f"es{j}_{qs}_{k}")
                nc.scalar.activation(es[:], pss[:], EXPF, scale=SCALE)
                if prev is not None:
                    emit_pv(k - 1, prev)
                prev = es
            emit_pv(KB - 1, prev)

            # denominators -> reciprocal -> replicate -> scale U rows
            for i, off in ((0, 0), (1, 64)):
                rb = rpool.tile([128, 512], F32, tag="rb",
                                name=f"rb{j}_{qs}_{i}")
                rc = rpool.tile([128, 512], F32, tag="rb",
                                name=f"rc{j}_{qs}_{i}")
                rg = rpool.tile([128, 8], F32, tag="rg",
                                name=f"rg{j}_{qs}_{i}")
                # denom row (512 on one partition) -> spread over 128
                # partitions so the exact reciprocal runs 4 elems/lane
                nc.vector.tensor_copy(rb[64:65, :], psu[i][64:65, :])
                nc.sync.dma_start(rg[:, 0:4], rb[64:65, :])
                nc.vector.reciprocal(rg[:, 4:8], rg[:, 0:4])
                nc.sync.dma_start(rc[0:1, :], rg[:, 4:8])
                nc.gpsimd.partition_broadcast(rc[0:64, :], rc[0:1, :])
                if off == 0:
                    nc.vector.tensor_mul(UT[j][0:64, q0:q0 + 512],
                                         psu[i][0:64, :], rc[0:64, :])
                else:
                    st = stpool.tile([64, 512], F32R, tag="st",
                                     name=f"st{j}_{qs}_{i}")
                    nc.vector.tensor_mul(st[0:64, :], psu[i][0:64, :],
                                         rc[0:64, :])
                    nc.sync.dma_start(UT[j][64:128, q0:q0 + 512],
                                      st[0:64, :])

        def proj_qs(qs):
            for qb in range(qs * 4, qs * 4 + 4):
                plo = psum_u.tile([128, 512], F32, tag="psu", name=f"pl{qb}")
                phi = psum_u.tile([128, 512], F32, tag="psu", name=f"ph{qb}")
                for m in range(MT):
                    lhsT = UT[m][:, qb * 128:(qb + 1) * 128]
                    nc.tensor.matmul(plo[:, 0:512], lhsT, wp_t[m][:, 0:512],
                                     start=(m == 0), stop=(m == MT - 1))
                    nc.tensor.matmul(phi[:, 0:256], lhsT,
                                     wp_t[m][:, 512:768],
                                     start=(m == 0), stop=(m == MT - 1))
                ot = opool.tile([128, C], F32, tag="ost", name=f"ot{qb}")
                nc.vector.tensor_copy(ot[:, 0:512], plo[:, 0:512])
                nc.vector.tensor_copy(ot[:, 512:768], phi[:, 0:256])
                nc.sync.dma_start(out[qb * 128:(qb + 1) * 128, :], ot[:])

        # ---- emission schedule: pipeline QKV m-tiles into attention ----
        VHEAD = min(6, KB)
        qk_mtile(0)
        v_proj(range(VHEAD))
        attn_pair(0, 0, k_hook=lambda k: v_tile(k) if k >= VHEAD else None)
        qk_mtile(1)
        attn_pair(0, 1)
        qk_mtile(2)
        attn_pair(0, 2)
        for qs in range(1, NQ):
            attn_pair(qs, 0)
            proj_qs(qs - 1)
            attn_pair(qs, 1)
            attn_pair(qs, 2)
        proj_qs(NQ - 1)

    nc.compile()
    return nc


_built = {}


def _get_nc(n_tok=2048):
    if n_tok not in _built:
        _built[n_tok] = build(n_tok)
    return _built[n_tok]


def make_in_maps(x, Wqkv, bqkv, Wproj):
    B, NT, _ = x.shape
    x = np.ascontiguousarray(np.asarray(x, dtype=np.float32))
    Wqkv = np.asarray(Wqkv, dtype=np.float32)
    bqkv = np.asarray(bqkv, dtype=np.float32)
    Wproj = np.asarray(Wproj, dtype=np.float32)
    in_maps = []
    for i in range(N_CORES):
        b, g = i // 2, i % 2
        s = g * CHG
        bq = bqkv[s:s + CHG].reshape(MT, 128).T
        bk = bqkv[C + s:C + s + CHG].reshape(MT, 128).T
        in_maps.append({
            "xT": np.ascontiguousarray(x[b].T),
            "wq": np.ascontiguousarray(Wqkv[:, s:s + CHG]),
            "wk": np.ascontiguousarray(Wqkv[:, C + s:C + s + CHG]),
            "wv": np.ascontiguousarray(Wqkv[:, 2 * C + s:2 * C + s + CHG]),
            "wp": np.ascontiguousarray(Wproj[s:s + CHG, :]),
            "bqk": np.ascontiguousarray(
                np.concatenate([bq, bk], axis=1)).astype(np.float32),
            "bv": np.ascontiguousarray(
                bqkv[2 * C + s:2 * C + s + CHG][None, :]).astype(np.float32),
        })
    return in_maps


def gather(results, bproj, B, NT):
    parts = [results[i]["out"] for i in range(N_CORES)]
    out = np.stack([parts[2 * b] + parts[2 * b + 1] for b in range(B)])
    return (out + np.asarray(bproj, np.float32)[None, None, :]).astype(np.float32)


def kernel(x, Wqkv, bqkv, Wproj, bproj, _trace=False):
    x = np.asarray(x)
    B, NT, _ = x.shape
    nc = _get_nc(NT)
    in_maps = make_in_maps(x, Wqkv, bqkv, Wproj)
    res = run_bass_kernel_spmd(nc, in_maps, core_ids=list(range(N_CORES)),
                               trace=_trace)
    out = gather(res.results, bproj, B, NT)
    if _trace:
        return out, res
    return out


# revision 36
# speedup vs baseline: 1.0077x; 1.0039x over previous
"""Multi-head attention block (12 heads, N=2048, C=768) on 8 NeuronCores.

Sharding: core i = (batch b = i//2, head-group g = i%2). Each core computes
attention for 6 heads of one batch plus its slice of the output projection
(row-sharded Wproj); the host sums the two head-group partials per batch.

Per-core dataflow:
  QKV projection and output projection run in float32r (full-rate fp32,
  ~1.6e-4 matmul error). Attention (scores / exp / attn@V) runs in bf16.
  xT [768,2048] arrives host-transposed; QT/KT [384,2048] are column-major
  (head h lives at partitions (h%2)*64..+64 of tile h//2), V2 is token-major
  with a ones column per head (66th col = pad for even free size).

  Heads are processed in pairs (a=2j at PE rows 0-63, b=2j+1 at rows 64-127).
  Per (pair, 512-query chunk qs, key block k):
    S^T_a -> pss[:, 0:512], S^T_b -> pss[:, 512:1024]  (two matmuls in
      disjoint PE row groups, sharing one PSUM tile so the second has no
      semaphore wait and the pair runs concurrently in the array)
    es = exp(S/8) for both heads in ONE ACT instruction (PSUM->SBUF, bf16)
    U'_a += V2_a[k]^T @ es[:, 0:512], U'_b += V2_b[k]^T @ es[:, 512:1024]
      (PSUM [66,512] accumulated over k; row 64 = softmax denominator via
      the ones column; software-pipelined one k behind the scores)
  U rows are scaled by 1/denominator (DVE reciprocal + gpsimd partition
  broadcast + fused DVE multiply) into UT [384,2048] (f32r); odd heads take
  a small DMA hop to land at partitions 64-127.
  out = UT^T-chunks @ Wproj_rows (f32r, PSUM-accumulated), DMA out.
"""

import numpy as np
from contextlib import ExitStack

import concourse.bass as bass
import concourse.tile as tile
from concourse import bacc, mybir
from concourse.bass_utils import run_bass_kernel_spmd

N_CORES = 8
C = 768          # model dim
HG = 6           # heads per core
D = 64           # head dim
CHG = HG * D     # 384, per-group qkv width
CC = C // 128    # 6 contraction chunks
MT = CHG // 128  # 3 m-tiles for QT/KT
SCALE = 1.0 / 8.0

F32 = mybir.dt.float32
F32R = mybir.dt.float32r
BF16 = mybir.dt.bfloat16


def build(n_tok: int = 2048):
    NT = n_tok
    KB = NT // 128           # key blocks
    NQ = NT // 512           # 512-wide query chunks
    EXPF = mybir.ActivationFunctionType.Exp

    nc = bacc.Bacc("TRN2", target_bir_lowering=False, debug=False,
                   num_devices=N_CORES)

    xT = nc.dram_tensor("xT", [C, NT], F32R, kind="ExternalInput").ap()
    wq = nc.dram_tensor("wq", [C, CHG], F32R, kind="ExternalInput").ap()
    wk = nc.dram_tensor("wk", [C, CHG], F32R, kind="ExternalInput").ap()
    wv = nc.dram_tensor("wv", [C, CHG], F32R, kind="ExternalInput").ap()
    wp = nc.dram_tensor("wp", [CHG, C], F32R, kind="ExternalInput").ap()
    bqk = nc.dram_tensor("bqk", [128, 2 * MT], F32, kind="ExternalInput").ap()
    bv = nc.dram_tensor("bv", [1, CHG], F32, kind="ExternalInput").ap()
    out = nc.dram_tensor("out", [NT, C], F32, kind="ExternalOutput").ap()

    with tile.TileContext(nc) as tc, ExitStack() as ctx:
        wpool = ctx.enter_context(tc.tile_pool(name="w", bufs=1))
        perm = ctx.enter_context(tc.tile_pool(name="perm", bufs=1))
        psum = ctx.enter_context(tc.tile_pool(name="ps", bufs=2, space="PSUM"))
        psum_u = ctx.enter_context(tc.tile_pool(name="psu", bufs=4,
                                                space="PSUM"))

        # ---- persistent SBUF ----
        wq_t = [wpool.tile([128, CHG], F32R, tag=f"wq{c}", name=f"wq{c}")
                for c in range(CC)]
        wk_t = [wpool.tile([128, CHG], F32R, tag=f"wk{c}", name=f"wk{c}")
                for c in range(CC)]
        wv_t = [wpool.tile([128, CHG], F32R, tag=f"wv{c}", name=f"wv{c}")
                for c in range(CC)]
        wp_t = [wpool.tile([128, C], F32R, tag=f"wp{m}", name=f"wp{m}")
                for m in range(MT)]
        bqk_t = wpool.tile([128, 2 * MT], F32, tag="bqk")
        bv_row = wpool.tile([1, CHG], F32, tag="bvr")
        bv_bc = wpool.tile([128, CHG], F32, tag="bvb")

        QT = [perm.tile([128, NT], BF16, tag=f"qt{m}", name=f"qtt{m}")
              for m in range(MT)]
        KT = [perm.tile([128, NT], BF16, tag=f"kt{m}", name=f"ktt{m}")
              for m in range(MT)]
        V2 = [perm.tile([128, HG, 66], BF16, tag=f"v2{t}", name=f"v2t{t}")
              for t in range(KB)]
        UT = [perm.tile([128, NT], F32R, tag=f"ut{m}", name=f"utt{m}")
              for m in range(MT)]

        # ---- input DMA ----
        for c in range(CC):
            nc.sync.dma_start(wq_t[c][:], wq[c * 128:(c + 1) * 128, :])
            nc.sync.dma_start(wk_t[c][:], wk[c * 128:(c + 1) * 128, :])
            nc.sync.dma_start(wv_t[c][:], wv[c * 128:(c + 1) * 128, :])
        for m in range(MT):
            nc.sync.dma_start(wp_t[m][:], wp[m * 128:(m + 1) * 128, :])
        nc.sync.dma_start(bqk_t[:], bqk)
        nc.sync.dma_start(bv_row[0:1, :], bv[0:1, :])
        nc.gpsimd.partition_broadcast(bv_bc[:], bv_row[0:1, :])
        for t in range(KB):
            nc.vector.tensor_scalar(
                V2[t][:, :, 64:66],
                bv_bc[:, 0:12].rearrange("p (a b) -> p a b", a=HG),
                0.0, 1.0, mybir.AluOpType.mult, mybir.AluOpType.add)

        spool = ctx.enter_context(tc.tile_pool(name="es", bufs=14))
        rpool = ctx.enter_context(tc.tile_pool(name="rb", bufs=4))
        stpool = ctx.enter_context(tc.tile_pool(name="st", bufs=3))
        opool = ctx.enter_context(tc.tile_pool(name="ost", bufs=3))
        xpool = ctx.enter_context(tc.tile_pool(name="xt", bufs=1))

        # ---- QKV projection pieces ----
        xt = []
        for c in range(CC):
            xc = xpool.tile([128, NT], F32R, tag=f"x{c}", name=f"xt{c}")
            nc.sync.dma_start(xc[:], xT[c * 128:(c + 1) * 128, :])
            xt.append(xc)

        def qk_mtile(m):
            for wt, dst, bcol in ((wq_t, QT, m), (wk_t, KT, MT + m)):
                for n in range(NQ):
                    ps = psum.tile([128, 512], F32, tag="ps",
                                   name=f"psqk{m}_{n}")
                    for c in range(CC):
                        nc.tensor.matmul(
                            ps[:], wt[c][:, m * 128:(m + 1) * 128],
                            xt[c][:, n * 512:(n + 1) * 512],
                            start=(c == 0), stop=(c == CC - 1))
                    nc.vector.tensor_scalar_add(
                        dst[m][:, n * 512:(n + 1) * 512], ps[:],
                        bqk_t[:, bcol:bcol + 1])

        def v_tile(t):
            ps = psum_u.tile([128, CHG], F32, tag="psu", name=f"psv{t}")
            for c in range(CC):
                nc.tensor.matmul(ps[:], xt[c][:, t * 128:(t + 1) * 128],
                                 wv_t[c][:],
                                 start=(c == 0), stop=(c == CC - 1))
            nc.vector.tensor_add(
                V2[t][:, :, 0:64],
                ps[:].rearrange("p (h d) -> p h d", h=HG),
                bv_bc[:].rearrange("p (h d) -> p h d", h=HG))

        def v_proj(ts):
            for t in ts:
                v_tile(t)

        # ---- attention pieces ----
        def attn_pair(qs, j, k_hook=None):
            ha, hb = 2 * j, 2 * j + 1
            q0 = qs * 512
            psu = [psum_u.tile([128, 512], F32, tag="psu",
                               name=f"psu{j}_{qs}_{i}") for i in range(2)]

            def emit_pv(k, es):
                for i, h in enumerate((ha, hb)):
                    nc.tensor.matmul(
                        psu[i][0:66, :], V2[k][:, h, :],
                        es[:, i * 512:(i + 1) * 512],
                        start=(k == 0), stop=(k == KB - 1))

            prev = None
            for k in range(KB):
                if k_hook is not None:
                    k_hook(k)
                pss = psum.tile([128, 1024], F32, tag="ps",
                                name=f"pss{j}_{qs}_{k}")
                # head a in PE rows 0-63 -> pss cols 0-511, head b in
                # rows 64-127 -> cols 512-1023; one shared PSUM tile so
                # the second matmul has no semaphore wait and the pair
                # runs concurrently in disjoint row groups.
                for i, off in ((0, 0), (1, 64)):
                    nc.tensor.matmul(
                        pss[:, i * 512:(i + 1) * 512],
                        KT[j][off:off + 64, k * 128:(k + 1) * 128],
                        QT[j][off:off + 64, q0:q0 + 512],
                        start=True, stop=True)
                es = spool.tile([128, 1024], BF16, tag="es",
                                name=f"es{j}_{qs}_{k}")
                nc.scalar.activation(es[:], pss[:], EXPF, scale=SCALE)
                if prev is not None:
                    emit_pv(k - 1, prev)
                prev = es
            emit_pv(KB - 1, prev)

            # denominators -> reciprocal -> replicate -> scale U rows
            for i, off in ((0, 0), (1, 64)):
                rb = rpool.tile([128, 512], F32, tag="rb",
                                name=f"rb{j}_{qs}_{i}")
                rc = rpool.tile([128, 512], F32, tag="rb",
                                name=f"rc{j}_{qs}_{i}")
                rg = rpool.tile([128, 8], F32, tag="rg",
                                name=f"rg{j}_{qs}_{i}")
                # denom row (512 on one partition) -> spread over 128
                # partitions so the exact reciprocal runs 4 elems/lane
                nc.vector.tensor_copy(rb[64:65, :], psu[i][64:65, :])
                nc.sync.dma_start(rg[:, 0:4], rb[64:65, :])
                nc.vector.reciprocal(rg[:, 4:8], rg[:, 0:4])
                nc.sync.dma_start(rc[0:1, :], rg[:, 4:8])
                nc.gpsimd.partition_broadcast(rc[0:64, :], rc[0:1, :])
                if off == 0:
                    nc.vector.tensor_mul(UT[j][0:64, q0:q0 + 512],
                                         psu[i][0:64, :], rc[0:64, :])
                else:
                    st = stpool.tile([64, 512], F32R, tag="st",
                                     name=f"st{j}_{qs}_{i}")
                    nc.vector.tensor_mul(st[0:64, :], psu[i][0:64, :],
                                         rc[0:64, :])
                    nc.sync.dma_start(UT[j][64:128, q0:q0 + 512],
                                      st[0:64, :])

        def proj_qs(qs):
            for qb in range(qs * 4, qs * 4 + 4):
                plo = psum_u.tile([128, 512], F32, tag="psu", name=f"pl{qb}")
                phi = psum_u.tile([128, 512], F32, tag="psu", name=f"ph{qb}")
                for m in range(MT):
                    lhsT = UT[m][:, qb * 128:(qb + 1) * 128]
                    nc.tensor.matmul(plo[:, 0:512], lhsT, wp_t[m][:, 0:512],
                                     start=(m == 0), stop=(m == MT - 1))
                    nc.tensor.matmul(phi[:, 0:256], lhsT,
                                     wp_t[m][:, 512:768],
                                     start=(m == 0), stop=(m == MT - 1))
                ot = opool.tile([128, C], F32, tag="ost", name=f"ot{qb}")
                nc.vector.tensor_copy(ot[:, 0:512], plo[:, 0:512])
                nc.vector.tensor_copy(ot[:, 512:768], phi[:, 0:256])
                nc.sync.dma_start(out[qb * 128:(qb + 1) * 128, :], ot[:])

        # ---- emission schedule: pipeline QKV m-tiles into attention ----
        VHEAD = min(6, KB)
        qk_mtile(0)
        v_proj(range(VHEAD))
        attn_pair(0, 0, k_hook=lambda k: v_tile(k) if k >= VHEAD else None)
        qk_mtile(1)
        attn_pair(0, 1)
        qk_mtile(2)
        attn_pair(0, 2)
        for qs in range(1, NQ):
            attn_pair(qs, 0)
            proj_qs(qs - 1)
            attn_pair(qs, 1)
            attn_pair(qs, 2)
        proj_qs(NQ - 1)

    nc.compile()
    return nc


_built = {}


def _get_nc(n_tok=2048):
    if n_tok not in _built:
        _built[n_tok] = build(n_tok)
    return _built[n_tok]


def make_in_maps(x, Wqkv, bqkv, Wproj):
    B, NT, _ = x.shape
    x = np.ascontiguousarray(np.asarray(x, dtype=np.float32))
    Wqkv = np.asarray(Wqkv, dtype=np.float32)
    bqkv = np.asarray(bqkv, dtype=np.float32)
    Wproj = np.asarray(Wproj, dtype=np.float32)
    in_maps = []
    for i in range(N_CORES):
        b, g = i // 2, i % 2
        s = g * CHG
        bq = bqkv[s:s + CHG].reshape(MT, 128).T
        bk = bqkv[C + s:C + s + CHG].reshape(MT, 128).T
        in_maps.append({
            "xT": np.ascontiguousarray(x[b].T),
            "wq": np.ascontiguousarray(Wqkv[:, s:s + CHG]),
            "wk": np.ascontiguousarray(Wqkv[:, C + s:C + s + CHG]),
            "wv": np.ascontiguousarray(Wqkv[:, 2 * C + s:2 * C + s + CHG]),
            "wp": np.ascontiguousarray(Wproj[s:s + CHG, :]),
            "bqk": np.ascontiguousarray(
                np.concatenate([bq, bk], axis=1)).astype(np.float32),
            "bv": np.ascontiguousarray(
                bqkv[2 * C + s:2 * C + s + CHG][None, :]).astype(np.float32),
        })
    return in_maps


def gather(results, bproj, B, NT):
    parts = [results[i]["out"] for i in range(N_CORES)]
    out = np.stack([parts[2 * b] + parts[2 * b + 1] for b in range(B)])
    return (out + np.asarray(bproj, np.float32)[None, None, :]).astype(np.float32)


def kernel(x, Wqkv, bqkv, Wproj, bproj, _trace=False):
    x = np.asarray(x)
    B, NT, _ = x.shape
    nc = _get_nc(NT)
    in_maps = make_in_maps(x, Wqkv, bqkv, Wproj)
    res = run_bass_kernel_spmd(nc, in_maps, core_ids=list(range(N_CORES)),
                               trace=_trace)
    out = gather(res.results, bproj, B, NT)
    if _trace:
        return out, res
    return out
